# revision 12
# baseline (speedup 1.0000x reference)
"""Distributed multi-head attention kernel for 8 TRN2 NeuronCores.

Module: B=2, N=2048, D_MODEL=1024, H=16, D_HEAD=64 attention with
arbitrary rotary embedding, key-side boolean masking, softmax, and
output projection.

Sharding: head-parallel attention (2 heads per core, both batches),
one combined AllToAll (~1 MB/core, bf16, no padding) to switch to
row-parallel for the output projection. Each core returns a
[512, 1024] row block.

v4 design:
 - Projections (phase 1) are EMITTED INTERLEAVED with the attention
   passes; the Tile scheduler fills the PE's idle time during the
   ACT-bound softmax stream with the next row-block's projection
   matmuls, hiding both the input-DMA wall and the batch-1
   projections.  PSUM: 2-bank projection ring + 4-bank score ring
   (div broadcast piggybacks on it) + 2 o-accumulator banks = 8.
 - Attention software-pipelined per 512-q-row pass: both heads'
   score blocks share one [128,1024] PSUM tile, one exp per key tile
   covers both heads, score MMs for kt+1 are emitted before attnV of
   kt so the PE never waits on the scalar engine.
 - Rotary on device: rot2(q) = ProtT.T @ q (constant +-1 permutation
   matmul) instead of host-rotated duplicate weight projections.
 - Softmax denominators via a ones-column in V (lhsT = [v | 1], M=65);
   key mask folded into the exp as a per-partition bias.
 - Per-pass normalization on the producing core: reciprocal_approx_fast
   on the o accumulator (row 64 = den), one bf16 K=1 broadcast matmul
   per head from partition 64, normalize numerators on DVE, ship
   normalized bf16 [64,512] rows.
 - ONE AllToAll over [8*128, 512]: slot j = this core's pass
   j=(b*4+qc) output; received shard j = core j's heads for my rows.
 - After the collective: a short junk-matmul burst (reading the landed
   data) re-warms the PE HAM clock before the 64 projection matmuls.
"""
import os
import warnings

warnings.filterwarnings("ignore")
import numpy as np
import ml_dtypes

from concourse import bacc, tile, mybir, bass_utils

B, N, DM, H, DH = 2, 2048, 1024, 16, 64
R = B * N
NCORES = 8
HPC = 2
CPC = HPC * DH       # 128 chans per core
KT = 8               # contraction tiles over d_model
RB = 8               # row blocks of 512 over R
NKEYT = 16           # key tiles of 128 over N
ROWS_PER_CORE = R // NCORES  # 512
QC = 512             # q rows per attention pass
NPASS = N // QC      # 4 passes per batch

F32 = mybir.dt.float32
BF16 = mybir.dt.bfloat16

SHARD_ROWS = CPC          # 128: [hA 64 | hB 64] (normalized, no dens)
VAUGW = 2 * (DH + 1)      # 130 cols per key tile: [vA | 1 | vB | 1]
N_REWARM = 18             # junk MMs to re-warm the PE clock post-A2A

LAST_EXEC_TIME_NS = None
LAST_TRACE_DIR = None


def _install_trace_shim():
    import sys
    import types
    import ctypes
    import contextlib

    if "antenv.axon_hooks" in sys.modules:
        return
    so_path = "/opt/axon/libaxon_pjrt.so"
    hook = None
    if os.path.exists(so_path):
        lib = ctypes.CDLL(so_path)
        if hasattr(lib, "axon_start_nrt_profile"):
            lib.axon_start_nrt_profile.argtypes = [
                ctypes.POINTER(ctypes.c_int64), ctypes.c_size_t]
            lib.axon_start_nrt_profile.restype = ctypes.c_int64
            lib.axon_stop_nrt_profile.argtypes = [ctypes.c_char_p]
            lib.axon_stop_nrt_profile.restype = ctypes.c_int64

            @contextlib.contextmanager
            def _hook(output_dir, device_ids):
                import jax
                jax.devices()
                if device_ids:
                    ids = (ctypes.c_int64 * len(device_ids))(*device_ids)
                    rc = lib.axon_start_nrt_profile(ids, len(device_ids))
                else:
                    rc = lib.axon_start_nrt_profile(None, 0)
                if rc != 0:
                    raise RuntimeError(f"axon_start_nrt_profile rc={rc}")
                try:
                    yield
                finally:
                    n = lib.axon_stop_nrt_profile(str(output_dir).encode())
                    print(f"[trace] {n} profile file(s) -> {output_dir}")

            hook = _hook

    mod = types.ModuleType("antenv.axon_hooks")
    mod.get_axon_ntff_profile_hook = lambda: hook
    mod.set_axon_ntff_profile_hook = lambda h: None
    sys.modules["antenv.axon_hooks"] = mod
    bass_utils.upload_artifacts = lambda tmpdir: tmpdir


def build(dbg=False):
    nc = bacc.Bacc("TRN2", target_bir_lowering=False, debug=False,
                   num_devices=NCORES)

    xt_d = nc.dram_tensor("xt", [DM, R], BF16, kind="ExternalInput")
    wq_d = nc.dram_tensor("wq", [DM, CPC], BF16, kind="ExternalInput")
    wk_d = nc.dram_tensor("wk", [DM, CPC], BF16, kind="ExternalInput")
    wv_d = nc.dram_tensor("wv", [DM, CPC], BF16, kind="ExternalInput")
    prot_d = nc.dram_tensor("prot", [128, 128], BF16, kind="ExternalInput")
    wout_d = nc.dram_tensor("wout", [DM, DM], BF16, kind="ExternalInput")
    boutb_d = nc.dram_tensor("boutb", [128, DM], F32, kind="ExternalInput")
    cost_d = nc.dram_tensor("cost", [CPC, N], BF16, kind="ExternalInput")
    sint_d = nc.dram_tensor("sint", [CPC, N], BF16, kind="ExternalInput")
    maskb_d = nc.dram_tensor("maskb", [128, R // 128], F32, kind="ExternalInput")
    vones_d = nc.dram_tensor("vones", [128, (R // 128) * 2], BF16,
                             kind="ExternalInput")

    out_d = nc.dram_tensor("out", [ROWS_PER_CORE, DM], F32, kind="ExternalOutput")

    a2a_in = nc.dram_tensor("a2a_in", [NCORES * SHARD_ROWS, ROWS_PER_CORE],
                            BF16)
    a2a_out = nc.dram_tensor("a2a_out", [NCORES * SHARD_ROWS, ROWS_PER_CORE],
                             BF16)

    scale = float(DH ** -0.5)

    with tile.TileContext(nc) as tc:
        with tc.tile_pool(name="persist", bufs=1) as pp:
            wq_sb = pp.tile([128, KT, CPC], BF16, tag="wq")
            wk_sb = pp.tile([128, KT, CPC], BF16, tag="wk")
            wv_sb = pp.tile([128, KT, CPC], BF16, tag="wv")
            prot_sb = pp.tile([128, 128], BF16, tag="prot")
            cost_sb = pp.tile([CPC, N], BF16, tag="cost")
            sint_sb = pp.tile([CPC, N], BF16, tag="sint")
            maskb_sb = pp.tile([128, R // 128], F32, tag="maskb")
            boutb_sb = pp.tile([128, DM], F32, tag="boutb")
            qt_sb = pp.tile([CPC, R], BF16, tag="qt")
            kt_sb = pp.tile([CPC, R], BF16, tag="kt")
            vaug_sb = pp.tile([128, (R // 128) * VAUGW], BF16, tag="vaug")
            wo_sb = pp.tile([128, KT, DM], BF16, tag="wo")
            ones_sb = pp.tile([128, 128], BF16, tag="ones")
            nc.vector.memset(ones_sb[:], 1.0)

            def ktview(d):
                return d.ap().rearrange("(k p) n -> p k n", p=128)

            xt_view = xt_d.ap().rearrange("(k p) n -> p k n", p=128)

            # first xt block + weights first so matmuls start early
            xt_sb0 = pp.tile([128, KT, 512], BF16, tag="xt0")
            for kt in range(KT):
                eng = nc.sync if kt % 2 == 0 else nc.scalar
                eng.dma_start(xt_sb0[:, kt, :], xt_view[:, kt, 0:512])
            nc.sync.dma_start(wq_sb[:], ktview(wq_d))
            nc.scalar.dma_start(wk_sb[:], ktview(wk_d))
            nc.gpsimd.dma_start(wv_sb[:], ktview(wv_d))
            nc.sync.dma_start(prot_sb[:], prot_d[:, :])
            # pre-load the ACT Exp table during the initial DMA wait
            warm_sb = pp.tile([1, 2], F32, tag="warm")
            nc.vector.memset(warm_sb[:], 0.0)
            nc.scalar.activation(warm_sb[0:1, 1:2], warm_sb[0:1, 0:1],
                                 mybir.ActivationFunctionType.Exp)
            nc.scalar.dma_start(cost_sb[:], cost_d[:, :])
            nc.gpsimd.dma_start(sint_sb[:], sint_d[:, :])
            nc.scalar.dma_start(maskb_sb[:], maskb_d[:, :])
            ones_view = vaug_sb[:].rearrange("p (t u w) -> p (t u) w",
                                             u=2, w=DH + 1)[:, :, DH]
            nc.gpsimd.dma_start(ones_view, vones_d[:, :])
            # wout + output bias: behind the phase-1 critical loads
            nc.scalar.dma_start(wo_sb[:], wout_d.ap().rearrange(
                "(k p) n -> p k n", p=128))
            nc.sync.dma_start(boutb_sb[:], boutb_d[:, :])

            with tc.tile_pool(name="p1", bufs=3) as p1, \
                 tc.tile_pool(name="psc", bufs=2, space="PSUM") as psc, \
                 tc.tile_pool(name="p2", bufs=3) as p2, \
                 tc.tile_pool(name="ps_sc", bufs=2, space="PSUM") as ps_sc, \
                 tc.tile_pool(name="ps_o", bufs=1, space="PSUM") as ps_o:

                def emit_rb(rb):
                    """Projection + rotary + v_aug for one 512-row block."""
                    c0 = rb * 512
                    if rb == 0:
                        xt_sb = xt_sb0
                    else:
                        xt_sb = p1.tile([128, KT, 512], BF16, tag="xt")
                        eng = [nc.sync, nc.scalar, nc.gpsimd][rb % 3]
                        eng.dma_start(xt_sb[:], xt_view[:, :, c0:c0 + 512])

                    q_ps = psc.tile([128, 512], F32, tag="c", name="q")
                    k_ps = psc.tile([128, 512], F32, tag="c", name="k")
                    for kt in range(KT):
                        st, sp = kt == 0, kt == KT - 1
                        nc.tensor.matmul(q_ps[:], wq_sb[:, kt, :],
                                         xt_sb[:, kt, :], start=st, stop=sp)
                        nc.tensor.matmul(k_ps[:], wk_sb[:, kt, :],
                                         xt_sb[:, kt, :], start=st, stop=sp)
                    qraw = p1.tile([128, 512], BF16, tag="qraw")
                    kraw = p1.tile([128, 512], BF16, tag="kraw")
                    nc.vector.tensor_copy(qraw[:], q_ps[:])
                    nc.vector.tensor_copy(kraw[:], k_ps[:])

                    v_ps = psc.tile([128, 512], F32, tag="c", name="v")
                    for kt in range(KT):
                        st, sp = kt == 0, kt == KT - 1
                        for vt in range(4):
                            nc.tensor.matmul(
                                v_ps[:, vt * 128:(vt + 1) * 128],
                                xt_sb[:, kt, vt * 128:(vt + 1) * 128],
                                wv_sb[:, kt, :], start=(st and vt == 0), stop=sp)
                    kt0 = rb * 4
                    va = vaug_sb[:].rearrange("p (t w) -> p t w", w=VAUGW)
                    vp = v_ps[:].rearrange("p (t c) -> p t c", c=128)
                    nc.vector.tensor_copy(va[:, kt0:kt0 + 4, 0:DH],
                                          vp[:, :, 0:DH])
                    nc.vector.tensor_copy(va[:, kt0:kt0 + 4, DH + 1:DH + 1 + DH],
                                          vp[:, :, DH:2 * DH])

                    cc = c0 % N
                    for dst, raw in [(qt_sb, qraw), (kt_sb, kraw)]:
                        rot_ps = psc.tile([128, 512], F32, tag="c", name="rot")
                        nc.tensor.matmul(rot_ps[:], prot_sb[:], raw[:],
                                         start=True, stop=True)
                        dv = dst[:, c0:c0 + 512]
                        tmp = p1.tile([128, 512], BF16, tag="rottmp")
                        nc.vector.tensor_mul(dv, raw[:], cost_sb[:, cc:cc + 512])
                        nc.vector.tensor_mul(tmp[:], rot_ps[:],
                                             sint_sb[:, cc:cc + 512])
                        nc.vector.tensor_add(dv, dv, tmp[:])

                def emit_pass(b, qc):
                    """One attention pass: 512 q rows, both heads."""
                    qb = b * N + qc * QC
                    j = b * NPASS + qc
                    o_ps = [ps_o.tile([DH + 1, QC], F32, tag=f"o{h}",
                                      name=f"o{h}") for h in range(HPC)]
                    pt_prev = None
                    for kt in range(NKEYT + 1):
                        if kt < NKEYT:
                            g = b * NKEYT + kt
                            krow = b * N + kt * 128
                            sc = ps_sc.tile([128, 2 * QC], F32, tag="sc",
                                            name="sc")
                            for h in range(HPC):
                                ho = h * DH
                                nc.tensor.matmul(
                                    sc[:, h * QC:(h + 1) * QC],
                                    kt_sb[ho:ho + DH, krow:krow + 128],
                                    qt_sb[ho:ho + DH, qb:qb + QC],
                                    start=True, stop=True)
                            pt = p2.tile([128, 2 * QC], BF16, tag="p",
                                         name="pt")
                            nc.scalar.activation(
                                pt[:], sc[:],
                                mybir.ActivationFunctionType.Exp,
                                bias=maskb_sb[:, g:g + 1], scale=scale)
                        if kt >= 1:
                            ktp = kt - 1
                            gp = b * NKEYT + ktp
                            for h in range(HPC):
                                va_l = vaug_sb[:, gp * VAUGW + h * (DH + 1):
                                               gp * VAUGW + (h + 1) * (DH + 1)]
                                nc.tensor.matmul(
                                    o_ps[h][:], va_l,
                                    pt_prev[:, h * QC:(h + 1) * QC],
                                    start=(ktp == 0), stop=(ktp == NKEYT - 1))
                        pt_prev = pt

                    # per-pass normalization on the producing core
                    for h in range(HPC):
                        rcp = p2.tile([DH + 1, QC], F32, tag=f"rcp{h}",
                                      name=f"rcp{h}")
                        nc.vector.reciprocal_approx_fast(rcp[:], o_ps[h][:])
                        rcpb = p2.tile([DH + 1, QC], BF16, tag=f"rb{h}",
                                       name=f"rb{h}")
                        nc.vector.tensor_copy(rcpb[DH:DH + 1, :],
                                              rcp[DH:DH + 1, :])
                        div_tile = ps_sc.tile([128, 2 * QC], F32, tag="sc",
                                              name="div")
                        div_ps = div_tile[:, 0:QC]
                        nc.tensor.matmul(div_ps, ones_sb[DH:DH + 1, :],
                                         rcpb[DH:DH + 1, :],
                                         start=True, stop=True,
                                         tile_position=(64, 0))
                        div_sb = p2.tile([DH, QC], BF16, tag=f"dv{h}",
                                         name=f"dv{h}")
                        nc.vector.tensor_copy(div_sb[:], div_ps[0:DH, :])
                        onb = p2.tile([DH, QC], BF16, tag=f"onb{h}",
                                      name=f"onb{h}")
                        nc.vector.tensor_mul(onb[:], o_ps[h][0:DH, :],
                                             div_sb[:])
                        r0 = j * SHARD_ROWS + h * DH
                        nc.gpsimd.dma_start(a2a_in[r0:r0 + DH, :], onb[:])

                # Emission order IS program order: every row block a pass
                # reads (its batch's full kt/vaug + its own qt block) must
                # be emitted before the pass.  Batch-1 row blocks are
                # emitted between batch-0 passes so their projections fill
                # the PE's idle time under the ACT-bound softmax stream.
                emit_rb(0)
                emit_rb(1)
                emit_rb(2)
                emit_rb(3)
                emit_pass(0, 0)
                emit_rb(4)
                emit_pass(0, 1)
                emit_rb(5)
                emit_pass(0, 2)
                emit_rb(6)
                emit_pass(0, 3)
                emit_rb(7)
                emit_pass(1, 0)
                emit_pass(1, 1)
                emit_pass(1, 2)
                emit_pass(1, 3)

                nc.gpsimd.collective_compute(
                    "AllToAll", mybir.AluOpType.bypass,
                    replica_groups=[list(range(NCORES))],
                    ins=[a2a_in.ap().opt()],
                    outs=[a2a_out.ap().opt()])

            # ---- Phase 3: gather + output projection ----
            with tc.tile_pool(name="p3", bufs=1) as p3, \
                 tc.tile_pool(name="p3b", bufs=2) as p3b, \
                 tc.tile_pool(name="psy", bufs=2, space="PSUM") as psy, \
                 tc.tile_pool(name="psbr", bufs=1, space="PSUM") as psbr:
                av = a2a_out.ap().rearrange("(j p) n -> p j n", p=SHARD_ROWS)
                o_t = p3.tile([128, NCORES, 512], BF16, tag="oall")
                nc.sync.dma_start(o_t[0:DH, :, :], av[0:DH, :, :])
                nc.scalar.dma_start(o_t[DH:CPC, :, :], av[DH:CPC, :, :])

                # re-warm the PE clock: a solid burst of back-to-back junk
                # MMs that depend on the landed A2A data, so they run right
                # when phase 3 becomes ready
                br_ps = psbr.tile([128, 512], F32, tag="bridge")
                for i in range(N_REWARM):
                    nc.tensor.matmul(br_ps[:], wq_sb[:, i % KT, :],
                                     o_t[:, 0, :],
                                     start=(i == 0), stop=(i == N_REWARM - 1))

                for rw in range(4):
                    y_ps = psy.tile([128, DM], F32, tag="y", name="y")
                    for j in range(NCORES):
                        st, sp = j == 0, j == NCORES - 1
                        for nb in range(2):
                            nc.tensor.matmul(
                                y_ps[:, nb * 512:(nb + 1) * 512],
                                o_t[:, j, rw * 128:(rw + 1) * 128],
                                wo_sb[:, j, nb * 512:(nb + 1) * 512],
                                start=st, stop=sp)
                    y_sb = p3b.tile([128, DM], F32, tag="y_sb")
                    nc.vector.tensor_add(y_sb[:], y_ps[:], boutb_sb[:])
                    eng = nc.sync if rw % 2 == 0 else nc.scalar
                    eng.dma_start(out_d[rw * 128:(rw + 1) * 128, :], y_sb[:])

    nc.compile()
    return nc


_NC_CACHE = None


def kernel(x, mask, pos_emb, Wq, Wkv, Wout, bout):
    global LAST_EXEC_TIME_NS, LAST_TRACE_DIR, _NC_CACHE

    x = np.asarray(x, dtype=np.float32)
    mask = np.asarray(mask)
    pos_emb = np.asarray(pos_emb, dtype=np.float32)
    Wq = np.asarray(Wq, dtype=np.float32)
    Wkv = np.asarray(Wkv, dtype=np.float32)
    Wout = np.asarray(Wout, dtype=np.float32)
    bout = np.asarray(bout, dtype=np.float32)

    bf = ml_dtypes.bfloat16
    xt = np.ascontiguousarray(x.reshape(R, DM).T).astype(bf)
    wk_full = Wkv[:, :H * DH]
    wv_full = Wkv[:, H * DH:]
    cost = np.ascontiguousarray(np.tile(np.cos(pos_emb).T, (HPC, 1))).astype(bf)
    sint = np.ascontiguousarray(np.tile(np.sin(pos_emb).T, (HPC, 1))).astype(bf)
    maskb = np.ascontiguousarray(
        np.where(mask.reshape(R), 0.0, -1e5).astype(np.float32)
        .reshape(R // 128, 128).T)
    boutb = np.ascontiguousarray(
        np.broadcast_to(bout[None, :], (128, DM)).astype(np.float32))
    # rot2 as a matmul: rot2(q) = P @ q (q in [chan, row] layout);
    # lhsT for the tensor engine is P.T
    prot = np.zeros((128, 128), dtype=bf)
    for i in range(64):
        prot[2 * i + 1, 2 * i] = -1.0
        prot[2 * i, 2 * i + 1] = 1.0

    in_maps = []
    for c in range(NCORES):
        cols = slice(c * CPC, (c + 1) * CPC)
        in_maps.append({
            "xt": xt,
            "wq": np.ascontiguousarray(Wq[:, cols]).astype(bf),
            "wk": np.ascontiguousarray(wk_full[:, cols]).astype(bf),
            "wv": np.ascontiguousarray(wv_full[:, cols]).astype(bf),
            "prot": prot,
            "wout": Wout.astype(bf),
            "boutb": boutb,
            "cost": cost,
            "sint": sint,
            "maskb": maskb,
            "vones": np.ones((128, (R // 128) * 2), dtype=bf),
        })

    dbg = bool(int(os.environ.get("BASS_KERNEL_DEBUG", "0")))
    if _NC_CACHE is None:
        _NC_CACHE = build(dbg=dbg)
    nc = _NC_CACHE

    trace = bool(int(os.environ.get("BASS_KERNEL_TRACE", "0")))
    kwargs = {}
    if trace:
        _install_trace_shim()
        tdir = os.environ.get("BASS_TRACE_DIR", "/tmp/bass_trace_out")
        import shutil
        shutil.rmtree(tdir, ignore_errors=True)
        os.makedirs(tdir, exist_ok=True)
        kwargs["tmpdir"] = tdir
    res = bass_utils.run_bass_kernel_spmd(
        nc, in_maps, core_ids=list(range(NCORES)), trace=trace, **kwargs)
    LAST_EXEC_TIME_NS = res.exec_time_ns
    if res.instructions_and_trace is not None:
        LAST_TRACE_DIR = res.instructions_and_trace[1]
        globals()["LAST_INSTS"] = res.instructions_and_trace[0]

    globals()["LAST_RESULTS"] = res.results
    y = np.concatenate([res.results[c]["out"] for c in range(NCORES)], axis=0)
    return y.reshape(B, N, DM)


# revision 17
# speedup vs baseline: 1.0582x; 1.0582x over previous
"""Distributed multi-head attention kernel for 8 TRN2 NeuronCores.

Module: B=2, N=2048, D_MODEL=1024, H=16, D_HEAD=64 attention with
arbitrary rotary embedding, key-side boolean masking, softmax, and
output projection.

Sharding: head-parallel attention (2 heads per core, both batches),
one combined AllToAll (~1 MB/core, bf16, no padding) to switch to
row-parallel for the output projection. Each core returns a
[512, 1024] row block.

v4 design:
 - Projections (phase 1) are EMITTED INTERLEAVED with the attention
   passes; the Tile scheduler fills the PE's idle time during the
   ACT-bound softmax stream with the next row-block's projection
   matmuls, hiding both the input-DMA wall and the batch-1
   projections.  PSUM: 2-bank projection ring + 4-bank score ring
   (div broadcast piggybacks on it) + 2 o-accumulator banks = 8.
 - Attention software-pipelined per 512-q-row pass: both heads'
   score blocks share one [128,1024] PSUM tile, one exp per key tile
   covers both heads, score MMs for kt+1 are emitted before attnV of
   kt so the PE never waits on the scalar engine.
 - Rotary on device: rot2(q) = ProtT.T @ q (constant +-1 permutation
   matmul) instead of host-rotated duplicate weight projections.
 - Softmax denominators via a ones-column in V (lhsT = [v | 1], M=65);
   key mask folded into the exp as a per-partition bias.
 - Per-pass normalization on the producing core: reciprocal_approx_fast
   on the o accumulator (row 64 = den), one bf16 K=1 broadcast matmul
   per head from partition 64, normalize numerators on DVE, ship
   normalized bf16 [64,512] rows.
 - ONE AllToAll over [8*128, 512]: slot j = this core's pass
   j=(b*4+qc) output; received shard j = core j's heads for my rows.
 - After the collective: a short junk-matmul burst (reading the landed
   data) re-warms the PE HAM clock before the 64 projection matmuls.
"""
import os
import warnings

warnings.filterwarnings("ignore")
import numpy as np
import ml_dtypes

from concourse import bacc, tile, mybir, bass_utils

B, N, DM, H, DH = 2, 2048, 1024, 16, 64
R = B * N
NCORES = 8
HPC = 2
CPC = HPC * DH       # 128 chans per core
KT = 8               # contraction tiles over d_model
RB = 8               # row blocks of 512 over R
NKEYT = 16           # key tiles of 128 over N
ROWS_PER_CORE = R // NCORES  # 512
QC = 512             # q rows per attention pass
NPASS = N // QC      # 4 passes per batch

F32 = mybir.dt.float32
BF16 = mybir.dt.bfloat16

SHARD_ROWS = CPC          # 128: [hA 64 | hB 64] (normalized, no dens)
VAUGW = 2 * (DH + 1)      # 130 cols per key tile: [vA | 1 | vB | 1]
N_REWARM = 18             # junk MMs to re-warm the PE clock post-A2A

LAST_EXEC_TIME_NS = None
LAST_TRACE_DIR = None


def _install_trace_shim():
    import sys
    import types
    import ctypes
    import contextlib

    if "antenv.axon_hooks" in sys.modules:
        return
    so_path = "/opt/axon/libaxon_pjrt.so"
    hook = None
    if os.path.exists(so_path):
        lib = ctypes.CDLL(so_path)
        if hasattr(lib, "axon_start_nrt_profile"):
            lib.axon_start_nrt_profile.argtypes = [
                ctypes.POINTER(ctypes.c_int64), ctypes.c_size_t]
            lib.axon_start_nrt_profile.restype = ctypes.c_int64
            lib.axon_stop_nrt_profile.argtypes = [ctypes.c_char_p]
            lib.axon_stop_nrt_profile.restype = ctypes.c_int64

            @contextlib.contextmanager
            def _hook(output_dir, device_ids):
                import jax
                jax.devices()
                if device_ids:
                    ids = (ctypes.c_int64 * len(device_ids))(*device_ids)
                    rc = lib.axon_start_nrt_profile(ids, len(device_ids))
                else:
                    rc = lib.axon_start_nrt_profile(None, 0)
                if rc != 0:
                    raise RuntimeError(f"axon_start_nrt_profile rc={rc}")
                try:
                    yield
                finally:
                    n = lib.axon_stop_nrt_profile(str(output_dir).encode())
                    print(f"[trace] {n} profile file(s) -> {output_dir}")

            hook = _hook

    mod = types.ModuleType("antenv.axon_hooks")
    mod.get_axon_ntff_profile_hook = lambda: hook
    mod.set_axon_ntff_profile_hook = lambda h: None
    sys.modules["antenv.axon_hooks"] = mod
    bass_utils.upload_artifacts = lambda tmpdir: tmpdir


def build(dbg=False):
    nc = bacc.Bacc("TRN2", target_bir_lowering=False, debug=False,
                   num_devices=NCORES)

    xt_d = nc.dram_tensor("xt", [DM, R], BF16, kind="ExternalInput")
    wq_d = nc.dram_tensor("wq", [DM, CPC], BF16, kind="ExternalInput")
    wk_d = nc.dram_tensor("wk", [DM, CPC], BF16, kind="ExternalInput")
    wv_d = nc.dram_tensor("wv", [DM, CPC], BF16, kind="ExternalInput")
    prot_d = nc.dram_tensor("prot", [128, 128], BF16, kind="ExternalInput")
    wout_d = nc.dram_tensor("wout", [DM, DM], BF16, kind="ExternalInput")
    boutb_d = nc.dram_tensor("boutb", [128, DM], F32, kind="ExternalInput")
    cost_d = nc.dram_tensor("cost", [CPC, N], BF16, kind="ExternalInput")
    sint_d = nc.dram_tensor("sint", [CPC, N], BF16, kind="ExternalInput")
    maskb_d = nc.dram_tensor("maskb", [128, R // 128], F32, kind="ExternalInput")
    vones_d = nc.dram_tensor("vones", [128, (R // 128) * 2], BF16,
                             kind="ExternalInput")

    out_d = nc.dram_tensor("out", [ROWS_PER_CORE, DM], F32, kind="ExternalOutput")

    a2a_in = nc.dram_tensor("a2a_in", [NCORES * SHARD_ROWS, ROWS_PER_CORE],
                            BF16)
    a2a_out = nc.dram_tensor("a2a_out", [NCORES * SHARD_ROWS, ROWS_PER_CORE],
                             BF16)

    scale = float(DH ** -0.5)

    with tile.TileContext(nc) as tc:
        with tc.tile_pool(name="persist", bufs=1) as pp:
            wq_sb = pp.tile([128, KT, CPC], BF16, tag="wq")
            wk_sb = pp.tile([128, KT, CPC], BF16, tag="wk")
            wv_sb = pp.tile([128, KT, CPC], BF16, tag="wv")
            prot_sb = pp.tile([128, 128], BF16, tag="prot")
            cost_sb = pp.tile([CPC, N], BF16, tag="cost")
            sint_sb = pp.tile([CPC, N], BF16, tag="sint")
            maskb_sb = pp.tile([128, R // 128], F32, tag="maskb")
            boutb_sb = pp.tile([128, DM], F32, tag="boutb")
            qt_sb = pp.tile([CPC, R], BF16, tag="qt")
            kt_sb = pp.tile([CPC, R], BF16, tag="kt")
            vaug_sb = pp.tile([128, (R // 128) * VAUGW], BF16, tag="vaug")
            wo_sb = pp.tile([128, KT, DM], BF16, tag="wo")
            ones_sb = pp.tile([128, 128], BF16, tag="ones")
            nc.vector.memset(ones_sb[:], 1.0)

            def ktview(d):
                return d.ap().rearrange("(k p) n -> p k n", p=128)

            xt_view = xt_d.ap().rearrange("(k p) n -> p k n", p=128)

            # first xt block + weights first so matmuls start early
            xt_sb0 = pp.tile([128, KT, 512], BF16, tag="xt0")
            for kt in range(KT):
                eng = nc.sync if kt % 2 == 0 else nc.scalar
                eng.dma_start(xt_sb0[:, kt, :], xt_view[:, kt, 0:512])
            nc.sync.dma_start(wq_sb[:], ktview(wq_d))
            nc.scalar.dma_start(wk_sb[:], ktview(wk_d))
            nc.gpsimd.dma_start(wv_sb[:], ktview(wv_d))
            nc.sync.dma_start(prot_sb[:], prot_d[:, :])
            # pre-load the ACT Exp table during the initial DMA wait
            warm_sb = pp.tile([1, 2], F32, tag="warm")
            nc.vector.memset(warm_sb[:], 0.0)
            nc.scalar.activation(warm_sb[0:1, 1:2], warm_sb[0:1, 0:1],
                                 mybir.ActivationFunctionType.Exp)
            nc.scalar.dma_start(cost_sb[:], cost_d[:, :])
            nc.gpsimd.dma_start(sint_sb[:], sint_d[:, :])
            nc.scalar.dma_start(maskb_sb[:], maskb_d[:, :])
            ones_view = vaug_sb[:].rearrange("p (t u w) -> p (t u) w",
                                             u=2, w=DH + 1)[:, :, DH]
            nc.gpsimd.dma_start(ones_view, vones_d[:, :])
            # wout + output bias: behind the phase-1 critical loads
            nc.scalar.dma_start(wo_sb[:], wout_d.ap().rearrange(
                "(k p) n -> p k n", p=128))
            nc.sync.dma_start(boutb_sb[:], boutb_d[:, :])

            with tc.tile_pool(name="p1", bufs=3) as p1, \
                 tc.tile_pool(name="psc", bufs=2, space="PSUM") as psc, \
                 tc.tile_pool(name="p2", bufs=3) as p2, \
                 tc.tile_pool(name="ps_sc", bufs=2, space="PSUM") as ps_sc, \
                 tc.tile_pool(name="ps_o", bufs=1, space="PSUM") as ps_o:

                XT_ENG = {1: nc.sync, 2: nc.scalar, 3: nc.sync,
                          4: nc.scalar, 5: nc.gpsimd, 6: nc.scalar,
                          7: nc.gpsimd}

                def rb_fillers(rb):
                    """Projection + rotary + v_aug for one 512-row block,
                    split into ~10 small chunks so they can be woven
                    between a pass's key-tile groups without starving the
                    scalar engine."""
                    c0 = rb * 512
                    st = {}

                    def f_start():
                        if rb == 0:
                            st['xt'] = xt_sb0
                        else:
                            st['xt'] = p1.tile([128, KT, 512], BF16, tag="xt", name="xt")
                            XT_ENG[rb].dma_start(st['xt'][:],
                                                 xt_view[:, :, c0:c0 + 512])
                        st['q'] = psc.tile([128, 512], F32, tag="c", name="q")

                    def f_q(k0):
                        def f():
                            for kt in range(k0, k0 + 4):
                                nc.tensor.matmul(
                                    st['q'][:], wq_sb[:, kt, :],
                                    st['xt'][:, kt, :],
                                    start=(kt == 0), stop=(kt == KT - 1))
                            if k0 + 4 == KT:
                                st['qraw'] = p1.tile([128, 512], BF16,
                                                     tag="qraw", name="qraw")
                                nc.vector.tensor_copy(st['qraw'][:],
                                                      st['q'][:])
                        return f

                    def f_k(k0):
                        def f():
                            if k0 == 0:
                                st['k'] = psc.tile([128, 512], F32, tag="c",
                                                   name="k")
                            for kt in range(k0, k0 + 4):
                                nc.tensor.matmul(
                                    st['k'][:], wk_sb[:, kt, :],
                                    st['xt'][:, kt, :],
                                    start=(kt == 0), stop=(kt == KT - 1))
                            if k0 + 4 == KT:
                                st['kraw'] = p1.tile([128, 512], BF16,
                                                     tag="kraw", name="kraw")
                                nc.vector.tensor_copy(st['kraw'][:],
                                                      st['k'][:])
                        return f

                    def f_v(k0):
                        def f():
                            if k0 == 0:
                                st['v'] = psc.tile([128, 512], F32, tag="c",
                                                   name="v")
                            for kt in range(k0, k0 + 4):
                                for vt in range(4):
                                    nc.tensor.matmul(
                                        st['v'][:, vt * 128:(vt + 1) * 128],
                                        st['xt'][:, kt, vt * 128:(vt + 1) * 128],
                                        wv_sb[:, kt, :],
                                        start=(kt == 0 and vt == 0),
                                        stop=(kt == KT - 1))
                            if k0 + 4 == KT:
                                kt0 = rb * 4
                                va = vaug_sb[:].rearrange("p (t w) -> p t w",
                                                          w=VAUGW)
                                vp = st['v'][:].rearrange("p (t c) -> p t c",
                                                          c=128)
                                nc.vector.tensor_copy(
                                    va[:, kt0:kt0 + 4, 0:DH], vp[:, :, 0:DH])
                                nc.vector.tensor_copy(
                                    va[:, kt0:kt0 + 4, DH + 1:DH + 1 + DH],
                                    vp[:, :, DH:2 * DH])
                        return f

                    def f_rot(dst, rawkey):
                        def f():
                            raw = st[rawkey]
                            rot_ps = psc.tile([128, 512], F32, tag="c",
                                              name="rot")
                            nc.tensor.matmul(rot_ps[:], prot_sb[:], raw[:],
                                             start=True, stop=True)
                            cc = c0 % N
                            dv = dst[:, c0:c0 + 512]
                            tmp = p1.tile([128, 512], BF16, tag="rottmp")
                            nc.vector.tensor_mul(dv, raw[:],
                                                 cost_sb[:, cc:cc + 512])
                            nc.vector.tensor_mul(tmp[:], rot_ps[:],
                                                 sint_sb[:, cc:cc + 512])
                            nc.vector.tensor_add(dv, dv, tmp[:])
                        return f

                    def f_first():
                        f_start()
                        f_q(0)()
                    return [f_first, f_q(4), f_k(0), f_k(4),
                            f_v(0), f_v(4),
                            f_rot(qt_sb, 'qraw'), f_rot(kt_sb, 'kraw')]

                def emit_rb(rb):
                    for f in rb_fillers(rb):
                        f()

                def emit_pass(b, qc, fillers=()):
                    """One attention pass: 512 q rows, both heads.
                    `fillers` are woven one per key-tile group so their PE
                    work interleaves with the ACT-bound exp stream."""
                    qb = b * N + qc * QC
                    j = b * NPASS + qc
                    o_ps = [ps_o.tile([DH + 1, QC], F32, tag=f"o{h}",
                                      name=f"o{h}") for h in range(HPC)]
                    pt_prev = None
                    for kt in range(NKEYT + 1):
                        if kt >= 1 and kt - 1 < len(fillers):
                            fillers[kt - 1]()
                        if kt < NKEYT:
                            g = b * NKEYT + kt
                            krow = b * N + kt * 128
                            sc = ps_sc.tile([128, 2 * QC], F32, tag="sc",
                                            name="sc")
                            for h in range(HPC):
                                ho = h * DH
                                nc.tensor.matmul(
                                    sc[:, h * QC:(h + 1) * QC],
                                    kt_sb[ho:ho + DH, krow:krow + 128],
                                    qt_sb[ho:ho + DH, qb:qb + QC],
                                    start=True, stop=True)
                            pt = p2.tile([128, 2 * QC], BF16, tag="p",
                                         name="pt")
                            nc.scalar.activation(
                                pt[:], sc[:],
                                mybir.ActivationFunctionType.Exp,
                                bias=maskb_sb[:, g:g + 1], scale=scale)
                        if kt >= 1:
                            ktp = kt - 1
                            gp = b * NKEYT + ktp
                            for h in range(HPC):
                                va_l = vaug_sb[:, gp * VAUGW + h * (DH + 1):
                                               gp * VAUGW + (h + 1) * (DH + 1)]
                                nc.tensor.matmul(
                                    o_ps[h][:], va_l,
                                    pt_prev[:, h * QC:(h + 1) * QC],
                                    start=(ktp == 0), stop=(ktp == NKEYT - 1))
                        pt_prev = pt

                    # per-pass normalization on the producing core
                    for h in range(HPC):
                        rcp = p2.tile([DH + 1, QC], F32, tag=f"rcp{h}",
                                      name=f"rcp{h}")
                        nc.vector.reciprocal_approx_fast(rcp[:], o_ps[h][:])
                        rcpb = p2.tile([DH + 1, QC], BF16, tag=f"rb{h}",
                                       name=f"rb{h}")
                        nc.vector.tensor_copy(rcpb[DH:DH + 1, :],
                                              rcp[DH:DH + 1, :])
                        div_tile = ps_sc.tile([128, 2 * QC], F32, tag="sc",
                                              name="div")
                        div_ps = div_tile[:, 0:QC]
                        nc.tensor.matmul(div_ps, ones_sb[DH:DH + 1, :],
                                         rcpb[DH:DH + 1, :],
                                         start=True, stop=True,
                                         tile_position=(64, 0))
                        div_sb = p2.tile([DH, QC], BF16, tag=f"dv{h}",
                                         name=f"dv{h}")
                        nc.vector.tensor_copy(div_sb[:], div_ps[0:DH, :])
                        onb = p2.tile([DH, QC], BF16, tag=f"onb{h}",
                                      name=f"onb{h}")
                        nc.vector.tensor_mul(onb[:], o_ps[h][0:DH, :],
                                             div_sb[:])
                        r0 = j * SHARD_ROWS + h * DH
                        nc.sync.dma_start(a2a_in[r0:r0 + DH, :], onb[:])

                # Emission order IS program order: every row block a pass
                # reads (its batch's full kt/vaug + its own qt block) must
                # be emitted before the pass.  Batch-1 row blocks are woven
                # INTO the batch-0 passes at key-tile granularity so their
                # projection matmuls fill the PE's idle time under the
                # ACT-bound softmax stream without ever starving it.
                emit_rb(0)
                emit_rb(1)
                emit_rb(2)
                emit_rb(3)
                emit_pass(0, 0, rb_fillers(4))
                emit_pass(0, 1, rb_fillers(5))
                emit_pass(0, 2, rb_fillers(6))
                emit_pass(0, 3, rb_fillers(7))
                emit_pass(1, 0)
                emit_pass(1, 1)
                emit_pass(1, 2)
                emit_pass(1, 3)

                nc.gpsimd.collective_compute(
                    "AllToAll", mybir.AluOpType.bypass,
                    replica_groups=[list(range(NCORES))],
                    ins=[a2a_in.ap().opt()],
                    outs=[a2a_out.ap().opt()])

            # ---- Phase 3: gather + output projection ----
            with tc.tile_pool(name="p3", bufs=1) as p3, \
                 tc.tile_pool(name="p3b", bufs=2) as p3b, \
                 tc.tile_pool(name="psy", bufs=2, space="PSUM") as psy, \
                 tc.tile_pool(name="psbr", bufs=1, space="PSUM") as psbr:
                av = a2a_out.ap().rearrange("(j p) n -> p j n", p=SHARD_ROWS)
                o_t = p3.tile([128, NCORES, 512], BF16, tag="oall")
                nc.sync.dma_start(o_t[0:DH, :, :], av[0:DH, :, :])
                nc.scalar.dma_start(o_t[DH:CPC, :, :], av[DH:CPC, :, :])

                # re-warm the PE clock: a solid burst of back-to-back junk
                # MMs that depend on the landed A2A data, so they run right
                # when phase 3 becomes ready
                br_ps = psbr.tile([128, 512], F32, tag="bridge")
                for i in range(N_REWARM):
                    nc.tensor.matmul(br_ps[:], wq_sb[:, i % KT, :],
                                     o_t[:, 0, :],
                                     start=(i == 0), stop=(i == N_REWARM - 1))

                for rw in range(4):
                    y_ps = psy.tile([128, DM], F32, tag="y", name="y")
                    for j in range(NCORES):
                        st, sp = j == 0, j == NCORES - 1
                        for nb in range(2):
                            nc.tensor.matmul(
                                y_ps[:, nb * 512:(nb + 1) * 512],
                                o_t[:, j, rw * 128:(rw + 1) * 128],
                                wo_sb[:, j, nb * 512:(nb + 1) * 512],
                                start=st, stop=sp)
                    y_sb = p3b.tile([128, DM], F32, tag="y_sb")
                    nc.vector.tensor_add(y_sb[:], y_ps[:], boutb_sb[:])
                    eng = nc.sync if rw % 2 == 0 else nc.scalar
                    eng.dma_start(out_d[rw * 128:(rw + 1) * 128, :], y_sb[:])

    nc.compile()
    return nc


_NC_CACHE = None


def kernel(x, mask, pos_emb, Wq, Wkv, Wout, bout):
    global LAST_EXEC_TIME_NS, LAST_TRACE_DIR, _NC_CACHE

    x = np.asarray(x, dtype=np.float32)
    mask = np.asarray(mask)
    pos_emb = np.asarray(pos_emb, dtype=np.float32)
    Wq = np.asarray(Wq, dtype=np.float32)
    Wkv = np.asarray(Wkv, dtype=np.float32)
    Wout = np.asarray(Wout, dtype=np.float32)
    bout = np.asarray(bout, dtype=np.float32)

    bf = ml_dtypes.bfloat16
    xt = np.ascontiguousarray(x.reshape(R, DM).T).astype(bf)
    wk_full = Wkv[:, :H * DH]
    wv_full = Wkv[:, H * DH:]
    cost = np.ascontiguousarray(np.tile(np.cos(pos_emb).T, (HPC, 1))).astype(bf)
    sint = np.ascontiguousarray(np.tile(np.sin(pos_emb).T, (HPC, 1))).astype(bf)
    maskb = np.ascontiguousarray(
        np.where(mask.reshape(R), 0.0, -1e5).astype(np.float32)
        .reshape(R // 128, 128).T)
    boutb = np.ascontiguousarray(
        np.broadcast_to(bout[None, :], (128, DM)).astype(np.float32))
    # rot2 as a matmul: rot2(q) = P @ q (q in [chan, row] layout);
    # lhsT for the tensor engine is P.T
    prot = np.zeros((128, 128), dtype=bf)
    for i in range(64):
        prot[2 * i + 1, 2 * i] = -1.0
        prot[2 * i, 2 * i + 1] = 1.0

    in_maps = []
    for c in range(NCORES):
        cols = slice(c * CPC, (c + 1) * CPC)
        in_maps.append({
            "xt": xt,
            "wq": np.ascontiguousarray(Wq[:, cols]).astype(bf),
            "wk": np.ascontiguousarray(wk_full[:, cols]).astype(bf),
            "wv": np.ascontiguousarray(wv_full[:, cols]).astype(bf),
            "prot": prot,
            "wout": Wout.astype(bf),
            "boutb": boutb,
            "cost": cost,
            "sint": sint,
            "maskb": maskb,
            "vones": np.ones((128, (R // 128) * 2), dtype=bf),
        })

    dbg = bool(int(os.environ.get("BASS_KERNEL_DEBUG", "0")))
    if _NC_CACHE is None:
        _NC_CACHE = build(dbg=dbg)
    nc = _NC_CACHE

    trace = bool(int(os.environ.get("BASS_KERNEL_TRACE", "0")))
    kwargs = {}
    if trace:
        _install_trace_shim()
        tdir = os.environ.get("BASS_TRACE_DIR", "/tmp/bass_trace_out")
        import shutil
        shutil.rmtree(tdir, ignore_errors=True)
        os.makedirs(tdir, exist_ok=True)
        kwargs["tmpdir"] = tdir
    res = bass_utils.run_bass_kernel_spmd(
        nc, in_maps, core_ids=list(range(NCORES)), trace=trace, **kwargs)
    LAST_EXEC_TIME_NS = res.exec_time_ns
    if res.instructions_and_trace is not None:
        LAST_TRACE_DIR = res.instructions_and_trace[1]
        globals()["LAST_INSTS"] = res.instructions_and_trace[0]

    globals()["LAST_RESULTS"] = res.results
    y = np.concatenate([res.results[c]["out"] for c in range(NCORES)], axis=0)
    return y.reshape(B, N, DM)


# revision 19
# speedup vs baseline: 1.2651x; 1.1956x over previous
"""Distributed multi-head attention kernel for 8 TRN2 NeuronCores.

Module: B=2, N=2048, D_MODEL=1024, H=16, D_HEAD=64 attention with
arbitrary rotary embedding, key-side boolean masking, softmax, and
output projection.

Sharding: head-parallel attention (2 heads per core, both batches),
one combined AllToAll (~1 MB/core, bf16, no padding) to switch to
row-parallel for the output projection. Each core returns a
[512, 1024] row block.

v4 design:
 - Projections (phase 1) are EMITTED INTERLEAVED with the attention
   passes; the Tile scheduler fills the PE's idle time during the
   ACT-bound softmax stream with the next row-block's projection
   matmuls, hiding both the input-DMA wall and the batch-1
   projections.  PSUM: 2-bank projection ring + 4-bank score ring
   (div broadcast piggybacks on it) + 2 o-accumulator banks = 8.
 - Attention software-pipelined per 512-q-row pass: both heads'
   score blocks share one [128,1024] PSUM tile, one exp per key tile
   covers both heads, score MMs for kt+1 are emitted before attnV of
   kt so the PE never waits on the scalar engine.
 - Rotary on device: rot2(q) = ProtT.T @ q (constant +-1 permutation
   matmul) instead of host-rotated duplicate weight projections.
 - Softmax denominators via a ones-column in V (lhsT = [v | 1], M=65);
   key mask folded into the exp as a per-partition bias.
 - Per-pass normalization on the producing core: reciprocal_approx_fast
   on the o accumulator (row 64 = den), one bf16 K=1 broadcast matmul
   per head from partition 64, normalize numerators on DVE, ship
   normalized bf16 [64,512] rows.
 - ONE AllToAll over [8*128, 512]: slot j = this core's pass
   j=(b*4+qc) output; received shard j = core j's heads for my rows.
 - After the collective: a short junk-matmul burst (reading the landed
   data) re-warms the PE HAM clock before the 64 projection matmuls.
"""
import os
import warnings

warnings.filterwarnings("ignore")
import numpy as np
import ml_dtypes

from concourse import bacc, tile, mybir, bass_utils

B, N, DM, H, DH = 2, 2048, 1024, 16, 64
R = B * N
NCORES = 8
HPC = 2
CPC = HPC * DH       # 128 chans per core
KT = 8               # contraction tiles over d_model
RB = 8               # row blocks of 512 over R
NKEYT = 16           # key tiles of 128 over N
ROWS_PER_CORE = R // NCORES  # 512
QC = 512             # q rows per attention pass
NPASS = N // QC      # 4 passes per batch

F32 = mybir.dt.float32
BF16 = mybir.dt.bfloat16

SHARD_ROWS = CPC          # 128: [hA 64 | hB 64] (normalized, no dens)
VAUGW = 2 * (DH + 1)      # 130 cols per key tile: [vA | 1 | vB | 1]
N_REWARM = 18             # junk MMs to re-warm the PE clock post-A2A

LAST_EXEC_TIME_NS = None
LAST_TRACE_DIR = None


def _install_trace_shim():
    import sys
    import types
    import ctypes
    import contextlib

    if "antenv.axon_hooks" in sys.modules:
        return
    so_path = "/opt/axon/libaxon_pjrt.so"
    hook = None
    if os.path.exists(so_path):
        lib = ctypes.CDLL(so_path)
        if hasattr(lib, "axon_start_nrt_profile"):
            lib.axon_start_nrt_profile.argtypes = [
                ctypes.POINTER(ctypes.c_int64), ctypes.c_size_t]
            lib.axon_start_nrt_profile.restype = ctypes.c_int64
            lib.axon_stop_nrt_profile.argtypes = [ctypes.c_char_p]
            lib.axon_stop_nrt_profile.restype = ctypes.c_int64

            @contextlib.contextmanager
            def _hook(output_dir, device_ids):
                import jax
                jax.devices()
                if device_ids:
                    ids = (ctypes.c_int64 * len(device_ids))(*device_ids)
                    rc = lib.axon_start_nrt_profile(ids, len(device_ids))
                else:
                    rc = lib.axon_start_nrt_profile(None, 0)
                if rc != 0:
                    raise RuntimeError(f"axon_start_nrt_profile rc={rc}")
                try:
                    yield
                finally:
                    n = lib.axon_stop_nrt_profile(str(output_dir).encode())
                    print(f"[trace] {n} profile file(s) -> {output_dir}")

            hook = _hook

    mod = types.ModuleType("antenv.axon_hooks")
    mod.get_axon_ntff_profile_hook = lambda: hook
    mod.set_axon_ntff_profile_hook = lambda h: None
    sys.modules["antenv.axon_hooks"] = mod
    bass_utils.upload_artifacts = lambda tmpdir: tmpdir


def build(dbg=False):
    nc = bacc.Bacc("TRN2", target_bir_lowering=False, debug=False,
                   num_devices=NCORES)

    xt_d = nc.dram_tensor("xt", [DM, R], BF16, kind="ExternalInput")
    wq_d = nc.dram_tensor("wq", [DM, CPC], BF16, kind="ExternalInput")
    wk_d = nc.dram_tensor("wk", [DM, CPC], BF16, kind="ExternalInput")
    wv_d = nc.dram_tensor("wv", [DM, CPC], BF16, kind="ExternalInput")
    prot_d = nc.dram_tensor("prot", [128, 128], BF16, kind="ExternalInput")
    wout_d = nc.dram_tensor("wout", [DM, DM], BF16, kind="ExternalInput")
    boutb_d = nc.dram_tensor("boutb", [128, DM], F32, kind="ExternalInput")
    cost_d = nc.dram_tensor("cost", [CPC, N], BF16, kind="ExternalInput")
    sint_d = nc.dram_tensor("sint", [CPC, N], BF16, kind="ExternalInput")
    maskb_d = nc.dram_tensor("maskb", [128, R // 128], F32, kind="ExternalInput")
    vones_d = nc.dram_tensor("vones", [128, (R // 128) * 2], BF16,
                             kind="ExternalInput")

    out_d = nc.dram_tensor("out", [ROWS_PER_CORE, DM], F32, kind="ExternalOutput")

    a2a_in = nc.dram_tensor("a2a_in", [NCORES * SHARD_ROWS, ROWS_PER_CORE],
                            BF16)
    a2a_out = nc.dram_tensor("a2a_out", [NCORES * SHARD_ROWS, ROWS_PER_CORE],
                             BF16)

    scale = float(DH ** -0.5)

    with tile.TileContext(nc) as tc:
        with tc.tile_pool(name="persist", bufs=1) as pp:
            wq_sb = pp.tile([128, KT, CPC], BF16, tag="wq")
            wk_sb = pp.tile([128, KT, CPC], BF16, tag="wk")
            wv_sb = pp.tile([128, KT, CPC], BF16, tag="wv")
            prot_sb = pp.tile([128, 128], BF16, tag="prot")
            cost_sb = pp.tile([CPC, N], BF16, tag="cost")
            sint_sb = pp.tile([CPC, N], BF16, tag="sint")
            maskb_sb = pp.tile([128, R // 128], F32, tag="maskb")
            boutb_sb = pp.tile([128, DM], F32, tag="boutb")
            qt_sb = pp.tile([CPC, R], BF16, tag="qt")
            kt_sb = pp.tile([CPC, R], BF16, tag="kt")
            vaug_sb = pp.tile([128, (R // 128) * VAUGW], BF16, tag="vaug")
            wo_sb = pp.tile([128, KT, DM], BF16, tag="wo")
            ones_sb = pp.tile([128, 128], BF16, tag="ones")
            nc.vector.memset(ones_sb[:], 1.0)

            def ktview(d):
                return d.ap().rearrange("(k p) n -> p k n", p=128)

            xt_view = xt_d.ap().rearrange("(k p) n -> p k n", p=128)

            # first xt block + weights first so matmuls start early
            xt_sb0 = pp.tile([128, KT, 512], BF16, tag="xt0")
            for kt in range(KT):
                eng = nc.sync if kt % 2 == 0 else nc.scalar
                eng.dma_start(xt_sb0[:, kt, :], xt_view[:, kt, 0:512])
            nc.sync.dma_start(wq_sb[:], ktview(wq_d))
            nc.scalar.dma_start(wk_sb[:], ktview(wk_d))
            nc.gpsimd.dma_start(wv_sb[:], ktview(wv_d))
            nc.sync.dma_start(prot_sb[:], prot_d[:, :])
            # pre-load the ACT Exp table during the initial DMA wait
            warm_sb = pp.tile([1, 2], F32, tag="warm")
            nc.vector.memset(warm_sb[:], 0.0)
            nc.scalar.activation(warm_sb[0:1, 1:2], warm_sb[0:1, 0:1],
                                 mybir.ActivationFunctionType.Exp)
            nc.scalar.dma_start(cost_sb[:], cost_d[:, :])
            nc.gpsimd.dma_start(sint_sb[:], sint_d[:, :])
            nc.scalar.dma_start(maskb_sb[:], maskb_d[:, :])
            ones_view = vaug_sb[:].rearrange("p (t u w) -> p (t u) w",
                                             u=2, w=DH + 1)[:, :, DH]
            nc.gpsimd.dma_start(ones_view, vones_d[:, :])
            # wout + output bias: behind the phase-1 critical loads
            nc.scalar.dma_start(wo_sb[:], wout_d.ap().rearrange(
                "(k p) n -> p k n", p=128))
            nc.sync.dma_start(boutb_sb[:], boutb_d[:, :])

            with tc.tile_pool(name="p1", bufs=3) as p1, \
                 tc.tile_pool(name="psc", bufs=2, space="PSUM") as psc, \
                 tc.tile_pool(name="p2", bufs=3) as p2, \
                 tc.tile_pool(name="ps_sc", bufs=2, space="PSUM") as ps_sc, \
                 tc.tile_pool(name="ps_o", bufs=1, space="PSUM") as ps_o:

                XT_ENG = {1: nc.sync, 2: nc.scalar, 3: nc.sync,
                          4: nc.scalar, 5: nc.gpsimd, 6: nc.scalar,
                          7: nc.gpsimd}

                def rb_fillers(rb):
                    """Projection + rotary + v_aug for one 512-row block,
                    split into ~10 small chunks so they can be woven
                    between a pass's key-tile groups without starving the
                    scalar engine."""
                    c0 = rb * 512
                    st = {}

                    def f_start():
                        if rb == 0:
                            st['xt'] = xt_sb0
                        else:
                            st['xt'] = p1.tile([128, KT, 512], BF16, tag="xt", name="xt")
                            XT_ENG[rb].dma_start(st['xt'][:],
                                                 xt_view[:, :, c0:c0 + 512])
                        st['q'] = psc.tile([128, 512], F32, tag="c", name="q")

                    def f_q(k0):
                        def f():
                            for kt in range(k0, k0 + 4):
                                nc.tensor.matmul(
                                    st['q'][:], wq_sb[:, kt, :],
                                    st['xt'][:, kt, :],
                                    start=(kt == 0), stop=(kt == KT - 1))
                            if k0 + 4 == KT:
                                st['qraw'] = p1.tile([128, 512], BF16,
                                                     tag="qraw", name="qraw")
                                nc.vector.tensor_copy(st['qraw'][:],
                                                      st['q'][:])
                        return f

                    def f_k(k0):
                        def f():
                            if k0 == 0:
                                st['k'] = psc.tile([128, 512], F32, tag="c",
                                                   name="k")
                            for kt in range(k0, k0 + 4):
                                nc.tensor.matmul(
                                    st['k'][:], wk_sb[:, kt, :],
                                    st['xt'][:, kt, :],
                                    start=(kt == 0), stop=(kt == KT - 1))
                            if k0 + 4 == KT:
                                st['kraw'] = p1.tile([128, 512], BF16,
                                                     tag="kraw", name="kraw")
                                nc.vector.tensor_copy(st['kraw'][:],
                                                      st['k'][:])
                        return f

                    def f_v(k0):
                        def f():
                            if k0 == 0:
                                st['v'] = psc.tile([128, 512], F32, tag="c",
                                                   name="v")
                            for kt in range(k0, k0 + 4):
                                for vt in range(4):
                                    nc.tensor.matmul(
                                        st['v'][:, vt * 128:(vt + 1) * 128],
                                        st['xt'][:, kt, vt * 128:(vt + 1) * 128],
                                        wv_sb[:, kt, :],
                                        start=(kt == 0 and vt == 0),
                                        stop=(kt == KT - 1))
                            if k0 + 4 == KT:
                                kt0 = rb * 4
                                va = vaug_sb[:].rearrange("p (t w) -> p t w",
                                                          w=VAUGW)
                                vp = st['v'][:].rearrange("p (t c) -> p t c",
                                                          c=128)
                                nc.vector.tensor_copy(
                                    va[:, kt0:kt0 + 4, 0:DH], vp[:, :, 0:DH])
                                nc.vector.tensor_copy(
                                    va[:, kt0:kt0 + 4, DH + 1:DH + 1 + DH],
                                    vp[:, :, DH:2 * DH])
                        return f

                    def f_rot(dst, rawkey):
                        def f():
                            raw = st[rawkey]
                            rot_ps = psc.tile([128, 512], F32, tag="c",
                                              name="rot")
                            nc.tensor.matmul(rot_ps[:], prot_sb[:], raw[:],
                                             start=True, stop=True)
                            cc = c0 % N
                            dv = dst[:, c0:c0 + 512]
                            tmp = p1.tile([128, 512], BF16, tag="rottmp")
                            nc.vector.tensor_mul(dv, raw[:],
                                                 cost_sb[:, cc:cc + 512])
                            nc.vector.tensor_mul(tmp[:], rot_ps[:],
                                                 sint_sb[:, cc:cc + 512])
                            nc.vector.tensor_add(dv, dv, tmp[:])
                        return f

                    def f_first():
                        f_start()
                        f_q(0)()
                    return [f_first, f_q(4), f_k(0), f_k(4),
                            f_v(0), f_v(4),
                            f_rot(qt_sb, 'qraw'), f_rot(kt_sb, 'kraw')]

                def emit_rb(rb):
                    for f in rb_fillers(rb):
                        f()

                def emit_pass(b, qc, fillers=()):
                    """One attention pass: 512 q rows, both heads.
                    `fillers` are woven one per key-tile group so their PE
                    work interleaves with the ACT-bound exp stream."""
                    qb = b * N + qc * QC
                    j = b * NPASS + qc
                    o_ps = [ps_o.tile([DH + 1, QC], F32, tag=f"o{h}",
                                      name=f"o{h}") for h in range(HPC)]
                    pt_prev = None
                    for kt in range(NKEYT + 1):
                        if kt >= 1 and kt - 1 < len(fillers):
                            fillers[kt - 1]()
                        if kt < NKEYT:
                            g = b * NKEYT + kt
                            krow = b * N + kt * 128
                            sc = ps_sc.tile([128, 2 * QC], F32, tag="sc",
                                            name="sc")
                            for h in range(HPC):
                                ho = h * DH
                                nc.tensor.matmul(
                                    sc[:, h * QC:(h + 1) * QC],
                                    kt_sb[ho:ho + DH, krow:krow + 128],
                                    qt_sb[ho:ho + DH, qb:qb + QC],
                                    start=True, stop=True)
                            pt = p2.tile([128, 2 * QC], BF16, tag="p",
                                         name="pt")
                            nc.scalar.activation(
                                pt[:], sc[:],
                                mybir.ActivationFunctionType.Exp,
                                bias=maskb_sb[:, g:g + 1], scale=scale)
                        if kt >= 1:
                            ktp = kt - 1
                            gp = b * NKEYT + ktp
                            for h in range(HPC):
                                va_l = vaug_sb[:, gp * VAUGW + h * (DH + 1):
                                               gp * VAUGW + (h + 1) * (DH + 1)]
                                nc.tensor.matmul(
                                    o_ps[h][:], va_l,
                                    pt_prev[:, h * QC:(h + 1) * QC],
                                    start=(ktp == 0), stop=(ktp == NKEYT - 1))
                        pt_prev = pt

                    # Per-pass normalization on the producing core.  The o
                    # banks are released by the recip + numerator-evac pair
                    # (both DVE) so the next pass's first attnV never
                    # stalls; the div broadcast uses the projection ring
                    # (psc), NOT the score ring, so it never gates the
                    # next pass's score matmuls or exps.
                    for h in range(HPC):
                        rcp = p2.tile([DH + 1, QC], F32, tag=f"rcp{h}",
                                      name=f"rcp{h}")
                        nc.vector.reciprocal_approx_fast(rcp[:], o_ps[h][:])
                        onum = p2.tile([DH, QC], BF16, tag=f"on{h}",
                                       name=f"on{h}")
                        nc.vector.tensor_copy(onum[:], o_ps[h][0:DH, :])
                        rcpb = p2.tile([DH + 1, QC], BF16, tag=f"rb{h}",
                                       name=f"rb{h}")
                        nc.vector.tensor_copy(rcpb[DH:DH + 1, :],
                                              rcp[DH:DH + 1, :])
                        div_ps = psc.tile([128, QC], F32, tag="c", name="div")
                        nc.tensor.matmul(div_ps[:], ones_sb[DH:DH + 1, :],
                                         rcpb[DH:DH + 1, :],
                                         start=True, stop=True,
                                         tile_position=(64, 0))
                        div_sb = p2.tile([DH, QC], BF16, tag=f"dv{h}",
                                         name=f"dv{h}")
                        nc.vector.tensor_copy(div_sb[:], div_ps[0:DH, :])
                        onb = p2.tile([DH, QC], BF16, tag=f"onb{h}",
                                      name=f"onb{h}")
                        nc.vector.tensor_mul(onb[:], onum[:], div_sb[:])
                        r0 = j * SHARD_ROWS + h * DH
                        nc.sync.dma_start(a2a_in[r0:r0 + DH, :], onb[:])

                # Emission order IS program order: every row block a pass
                # reads (its batch's full kt/vaug + its own qt block) must
                # be emitted before the pass.  Batch-1 row blocks are woven
                # INTO the batch-0 passes at key-tile granularity so their
                # projection matmuls fill the PE's idle time under the
                # ACT-bound softmax stream without ever starving it.
                emit_rb(0)
                emit_rb(1)
                emit_rb(2)
                emit_rb(3)
                emit_pass(0, 0, rb_fillers(4))
                emit_pass(0, 1, rb_fillers(5))
                emit_pass(0, 2, rb_fillers(6))
                emit_pass(0, 3, rb_fillers(7))
                emit_pass(1, 0)
                emit_pass(1, 1)
                emit_pass(1, 2)
                emit_pass(1, 3)

                nc.gpsimd.collective_compute(
                    "AllToAll", mybir.AluOpType.bypass,
                    replica_groups=[list(range(NCORES))],
                    ins=[a2a_in.ap().opt()],
                    outs=[a2a_out.ap().opt()])

            # ---- Phase 3: gather + output projection ----
            with tc.tile_pool(name="p3", bufs=1) as p3, \
                 tc.tile_pool(name="p3b", bufs=2) as p3b, \
                 tc.tile_pool(name="psy", bufs=2, space="PSUM") as psy:
                av = a2a_out.ap().rearrange("(j p) n -> p j n", p=SHARD_ROWS)
                o_t = p3.tile([128, NCORES, 512], BF16, tag="oall")
                # split the gather by output row block so each projection
                # chain starts as soon as its slice lands
                for rw in range(4):
                    cs = slice(rw * 128, (rw + 1) * 128)
                    eng = nc.sync if rw % 2 == 0 else nc.scalar
                    eng.dma_start(o_t[:, :, cs], av[0:CPC, :, cs])

                for rw in range(4):
                    y_ps = psy.tile([128, DM], F32, tag="y", name="y")
                    for j in range(NCORES):
                        st, sp = j == 0, j == NCORES - 1
                        for nb in range(2):
                            nc.tensor.matmul(
                                y_ps[:, nb * 512:(nb + 1) * 512],
                                o_t[:, j, rw * 128:(rw + 1) * 128],
                                wo_sb[:, j, nb * 512:(nb + 1) * 512],
                                start=st, stop=sp)
                    y_sb = p3b.tile([128, DM], F32, tag="y_sb")
                    nc.vector.tensor_add(y_sb[:], y_ps[:], boutb_sb[:])
                    eng = nc.sync if rw % 2 == 0 else nc.scalar
                    eng.dma_start(out_d[rw * 128:(rw + 1) * 128, :], y_sb[:])

    nc.compile()
    return nc


_NC_CACHE = None


def kernel(x, mask, pos_emb, Wq, Wkv, Wout, bout):
    global LAST_EXEC_TIME_NS, LAST_TRACE_DIR, _NC_CACHE

    x = np.asarray(x, dtype=np.float32)
    mask = np.asarray(mask)
    pos_emb = np.asarray(pos_emb, dtype=np.float32)
    Wq = np.asarray(Wq, dtype=np.float32)
    Wkv = np.asarray(Wkv, dtype=np.float32)
    Wout = np.asarray(Wout, dtype=np.float32)
    bout = np.asarray(bout, dtype=np.float32)

    bf = ml_dtypes.bfloat16
    xt = np.ascontiguousarray(x.reshape(R, DM).T).astype(bf)
    wk_full = Wkv[:, :H * DH]
    wv_full = Wkv[:, H * DH:]
    cost = np.ascontiguousarray(np.tile(np.cos(pos_emb).T, (HPC, 1))).astype(bf)
    sint = np.ascontiguousarray(np.tile(np.sin(pos_emb).T, (HPC, 1))).astype(bf)
    maskb = np.ascontiguousarray(
        np.where(mask.reshape(R), 0.0, -1e5).astype(np.float32)
        .reshape(R // 128, 128).T)
    boutb = np.ascontiguousarray(
        np.broadcast_to(bout[None, :], (128, DM)).astype(np.float32))
    # rot2 as a matmul: rot2(q) = P @ q (q in [chan, row] layout);
    # lhsT for the tensor engine is P.T
    prot = np.zeros((128, 128), dtype=bf)
    for i in range(64):
        prot[2 * i + 1, 2 * i] = -1.0
        prot[2 * i, 2 * i + 1] = 1.0

    in_maps = []
    for c in range(NCORES):
        cols = slice(c * CPC, (c + 1) * CPC)
        in_maps.append({
            "xt": xt,
            "wq": np.ascontiguousarray(Wq[:, cols]).astype(bf),
            "wk": np.ascontiguousarray(wk_full[:, cols]).astype(bf),
            "wv": np.ascontiguousarray(wv_full[:, cols]).astype(bf),
            "prot": prot,
            "wout": Wout.astype(bf),
            "boutb": boutb,
            "cost": cost,
            "sint": sint,
            "maskb": maskb,
            "vones": np.ones((128, (R // 128) * 2), dtype=bf),
        })

    dbg = bool(int(os.environ.get("BASS_KERNEL_DEBUG", "0")))
    if _NC_CACHE is None:
        _NC_CACHE = build(dbg=dbg)
    nc = _NC_CACHE

    trace = bool(int(os.environ.get("BASS_KERNEL_TRACE", "0")))
    kwargs = {}
    if trace:
        _install_trace_shim()
        tdir = os.environ.get("BASS_TRACE_DIR", "/tmp/bass_trace_out")
        import shutil
        shutil.rmtree(tdir, ignore_errors=True)
        os.makedirs(tdir, exist_ok=True)
        kwargs["tmpdir"] = tdir
    res = bass_utils.run_bass_kernel_spmd(
        nc, in_maps, core_ids=list(range(NCORES)), trace=trace, **kwargs)
    LAST_EXEC_TIME_NS = res.exec_time_ns
    if res.instructions_and_trace is not None:
        LAST_TRACE_DIR = res.instructions_and_trace[1]
        globals()["LAST_INSTS"] = res.instructions_and_trace[0]

    globals()["LAST_RESULTS"] = res.results
    y = np.concatenate([res.results[c]["out"] for c in range(NCORES)], axis=0)
    return y.reshape(B, N, DM)
